# revision 12
# baseline (speedup 1.0000x reference)
"""Trainium2 Bass kernel for Llama-style attention (GQA 32q/8kv, RoPE,
non-causal softmax) on 8 NeuronCores — token-sharded design.

Each core owns a 512-token slice (cores 0-3 = batch 0, 4-7 = batch 1) and
computes ALL heads for its tokens:
  phase KV: kT (RoPE'd, [hd, tok]) + v ([tok, hd]) for the local slice,
            all 8 kv heads, written to all 8 slots of an AllToAll input
            (the gather is emulated with a full-group AllToAll because
            AllGather is ~5x slower on this runtime and AllToAll rejects
            groups of 4). Receivers combine block j of the two batch
            halves as out[j]*s0 + out[j+4]*s1 with per-core 0/1 scalars
            (bsel input), in place, on the Pool engine.
  phase Q:  qT for all 32 heads of the local slice (~160us of PE work that
            hides the AllToAll); RoPE on DVE.
  attention: per kv-group: stream+combine gathered K/V, then 4 q-heads of
            flash-style scoresT->exp->PV; softmax denominator partial adds
            on Pool, tree+normalize on DVE, partition reduce on GPSIMD.
            The last 8 q-head projections are interleaved between kv-groups
            to fill the PE idle created by the ACT-bound exp stream.
  o_proj:   fully local (all heads on-core): [512, 4096] x wo with wo
            streamed in 512-col chunks. No second collective.

All matmuls bf16 with fp32 PSUM. RoPE even/odd pairs become contiguous
64-partition blocks via host-side column permutation of wq/wk (scores are
invariant to a shared q/k head-dim permutation).
"""

import math
from contextlib import ExitStack
from dataclasses import dataclass

import numpy as np
import ml_dtypes

import concourse.bass as bass
import concourse.bass_isa as bass_isa
import concourse.mybir as mybir
import concourse.tile as tile
from concourse import bacc

BF16 = mybir.dt.bfloat16
F32 = mybir.dt.float32
AF = mybir.ActivationFunctionType
MUL = mybir.AluOpType.mult
ADD = mybir.AluOpType.add


@dataclass(frozen=True)
class Cfg:
    B: int = 2
    T: int = 2048          # sequence length per batch
    D: int = 4096          # model dim
    H: int = 32            # query heads
    HKV: int = 8           # kv heads
    HD: int = 128          # head dim
    NC: int = 8            # cores
    TLOC: int = 512        # tokens per core
    SGRP: int = 2          # s-tiles per scores psum group

    @property
    def KD(self):
        return self.D // 128          # contraction tiles over D

    @property
    def GSZ(self):
        return self.NC // self.B      # cores per batch group

    @property
    def NST(self):
        return self.T // 128          # s-tiles per batch (16)

    @property
    def GPH(self):
        return self.H // self.HKV     # q heads per kv head (4)


FULL = Cfg()


def build_nc(cfg: Cfg = FULL, collective: bool = True) -> bass.Bass:
    B, T, D, HD, NC = cfg.B, cfg.T, cfg.D, cfg.HD, cfg.NC
    H, HKV, KD, TLOC = cfg.H, cfg.HKV, cfg.KD, cfg.TLOC
    GSZ, NST, GPH, SGRP = cfg.GSZ, cfg.NST, cfg.GPH, cfg.SGRP
    NSG = NST // SGRP                 # scores groups per head (8)

    nc = bacc.Bacc(
        "TRN2",
        target_bir_lowering=False,
        debug=False,
        num_devices=NC,
    )

    # ---- per-core kernel I/O ----
    xT = nc.declare_dram_parameter("xT", [D, TLOC], BF16, isOutput=False)
    wq = nc.declare_dram_parameter("wq", [D, H * HD], BF16, isOutput=False)
    wk = nc.declare_dram_parameter("wk", [D, HKV * HD], BF16, isOutput=False)
    wv = nc.declare_dram_parameter("wv", [D, HKV * HD], BF16, isOutput=False)
    wo = nc.declare_dram_parameter("wo", [H * HD, D], BF16, isOutput=False)
    cosT = nc.declare_dram_parameter("cosT", [64, TLOC], F32, isOutput=False)
    sinT = nc.declare_dram_parameter("sinT", [64, TLOC], F32, isOutput=False)
    bsel = nc.declare_dram_parameter("bsel", [128, 2], F32, isOutput=False)
    out = nc.declare_dram_parameter("out", [TLOC, D], F32, isOutput=True)

    xT_v = xT.rearrange("(ko p) t -> p ko t", p=128)
    wq_v = wq.rearrange("(ko p) m -> p ko m", p=128)
    wk_v = wk.rearrange("(ko p) m -> p ko m", p=128)
    wv_v = wv.rearrange("(ko p) m -> p ko m", p=128)
    wo_v = wo.rearrange("(ko p) d -> p ko d", p=128)

    scale = 1.0 / math.sqrt(HD)

    with ExitStack() as ctx:
        tc = ctx.enter_context(tile.TileContext(nc))

        per = ctx.enter_context(tc.tile_pool(name="per", bufs=1))
        dram = ctx.enter_context(tc.tile_pool(name="dram", bufs=1, space="DRAM"))

        cos_sb = per.tile([64, TLOC], F32)
        sin_sb = per.tile([64, TLOC], F32)
        bsel_sb = per.tile([128, 2], F32)
        oT_sb = per.tile([128, H, TLOC], BF16)   # attention out, all heads

        # exchange: full-group AllGather (2MB up, 16MB down) with the
        # output in the "Shared" DRAM address space (fast HBM-HBM path;
        # sub-groups of 4 are rejected). Receivers pick their batch's four
        # blocks via the bsel combine.
        kv_in = dram.tile([2 * HKV, 128, TLOC], BF16)
        kv_out = dram.tile([NC, 2 * HKV, 128, TLOC], BF16,
                           addr_space="Shared")
        # views for merged gather reads: [p, j, slot, t] and v split [w c]
        kv_gk = kv_out.rearrange("j s p t -> p j s t")
        kv_gv = kv_out.rearrange("j s p (w c) -> p j w s c", c=128)

        def rope_apply(rope_pool, dst, psum):
            """psum [128, TLOC] f32 (evens parts 0:64, odds 64:128) ->
            dst bf16, RoPE'd with the local cos/sin slice."""
            c = cos_sb[:]
            s = sin_sb[:]
            qe = psum[0:64, :]
            qo = psum[64:128, :]
            t0 = rope_pool.tile([64, TLOC], F32, tag="rp0")
            t1 = rope_pool.tile([64, TLOC], F32, tag="rp1")
            nc.vector.tensor_mul(t0[:], qe, c)
            nc.vector.tensor_mul(t1[:], qo, s)
            nc.vector.tensor_sub(dst[0:64, :], t0[:], t1[:])
            t2 = rope_pool.tile([64, TLOC], F32, tag="rp0")
            t3 = rope_pool.tile([64, TLOC], F32, tag="rp1")
            nc.vector.tensor_mul(t2[:], qe, s)
            nc.vector.tensor_mul(t3[:], qo, c)
            nc.vector.tensor_add(dst[64:128, :], t2[:], t3[:])

        with ExitStack() as ctx_a:
            xt_pool = ctx_a.enter_context(tc.tile_pool(name="xt", bufs=1))
            rope_pool = ctx_a.enter_context(tc.tile_pool(name="rope", bufs=3))
            pproj = ctx_a.enter_context(
                tc.tile_pool(name="pproj", bufs=2, space="PSUM"))

            xt = xt_pool.tile([128, KD, TLOC], BF16)

            # ---------- phase KV ----------
            with tc.tile_pool(name="wkv", bufs=2) as wkv_pool, \
                 tc.tile_pool(name="kvloc", bufs=1) as kvloc:
                kT_loc = kvloc.tile([128, HKV, TLOC], BF16)
                v_loc = kvloc.tile([128, TLOC // 128, HKV * HD], BF16)

                # first k-head weights first so PE starts ASAP, then x.
                wk0 = wkv_pool.tile([128, KD, HD], BF16, tag="wkh", bufs=4)
                nc.sync.dma_start(wk0[:], wk_v[:, :, 0:HD])
                for kg in [(0, 4), (4, 8), (8, 16), (16, 24), (24, 32)]:
                    nc.sync.dma_start(xt[:, kg[0]:kg[1], :],
                                      xT_v[:, kg[0]:kg[1], :])
                nc.sync.dma_start(cos_sb[:], cosT[:])
                nc.sync.dma_start(sin_sb[:], sinT[:])
                nc.sync.dma_start(bsel_sb[:], bsel[:])

                # rolling 4-deep prefetch of per-head k weights
                wk_tiles = [wk0]
                for h in range(1, 4):
                    t = wkv_pool.tile([128, KD, HD], BF16, tag="wkh",
                                      bufs=4, name=f"wk{h}")
                    nc.sync.dma_start(t[:], wk_v[:, :, h * HD:(h + 1) * HD])
                    wk_tiles.append(t)
                for h in range(HKV):
                    if h + 4 < HKV:
                        t = wkv_pool.tile([128, KD, HD], BF16, tag="wkh",
                                          bufs=4, name=f"wk{h + 4}")
                        nc.sync.dma_start(
                            t[:], wk_v[:, :, (h + 4) * HD:(h + 5) * HD])
                        wk_tiles.append(t)
                    wk_sb = wk_tiles[h]
                    pk = pproj.tile([128, TLOC], F32, tag="pp")
                    for k in range(KD):
                        nc.tensor.matmul(
                            pk[:], lhsT=wk_sb[:, k, :], rhs=xt[:, k, :],
                            start=(k == 0), stop=(k == KD - 1),
                        )
                    rope_apply(rope_pool, kT_loc[:, h, :], pk)
                    nc.sync.dma_start(kv_in[h], kT_loc[:, h, :])

                # V natural: psum [s-tile 128, 512 cols = 4 heads]; both
                # weight halves prefetched so the PE never waits on them.
                wv_sbs = []
                for vh in range(2):
                    wv_sb = wkv_pool.tile([128, KD, 512], BF16, tag="wvh")
                    for kg in range(0, KD, 16):
                        nc.sync.dma_start(
                            wv_sb[:, kg:kg + 16, :],
                            wv_v[:, kg:kg + 16, vh * 512:(vh + 1) * 512])
                    wv_sbs.append(wv_sb)
                for vh in range(2):            # column halves of wv
                    wv_sb = wv_sbs[vh]
                    for st in range(TLOC // 128):
                        pv = pproj.tile([128, 512], F32, tag="pp")
                        for k in range(KD):
                            nc.tensor.matmul(
                                pv[:],
                                lhsT=xt[:, k, st * 128:(st + 1) * 128],
                                rhs=wv_sb[:, k, :],
                                start=(k == 0), stop=(k == KD - 1),
                            )
                        nc.vector.tensor_copy(
                            v_loc[:, st, vh * 512:(vh + 1) * 512], pv[:])
                for h in range(HKV):
                    nc.sync.dma_start(
                        kv_in[HKV + h],
                        v_loc[:, :, h * HD:(h + 1) * HD])

            if collective:
                nc.gpsimd.collective_compute(
                    "AllGather",
                    mybir.AluOpType.bypass,
                    replica_groups=[list(range(NC))],
                    ins=[kv_in.opt()],
                    outs=[kv_out.opt()],
                )
            else:
                # timing-only stand-in (results wrong): Shared DRAM allows a
                # single writer instruction, so copy one block.
                nc.sync.dma_start(kv_out[0], kv_in[:])

            # ---------- phase Q + attention (interleaved) ----------
            with ExitStack() as ctx_b:
                wqp = ctx_b.enter_context(tc.tile_pool(name="wqp", bufs=3))
                qT_pool = ctx_b.enter_context(tc.tile_pool(name="qT", bufs=1))
                kva = ctx_b.enter_context(tc.tile_pool(name="kva", bufs=2))
                pe_ps = ctx_b.enter_context(
                    tc.tile_pool(name="pe_ps", bufs=2, space="PSUM"))
                po_ps = ctx_b.enter_context(
                    tc.tile_pool(name="po_ps", bufs=2, space="PSUM"))
                epool = ctx_b.enter_context(tc.tile_pool(name="e", bufs=4))
                dpool = ctx_b.enter_context(tc.tile_pool(name="den", bufs=1))

                qT_sb = qT_pool.tile([128, H, TLOC], BF16)
                s0 = bsel_sb[:, 0:1]
                s1 = bsel_sb[:, 1:2]

                def qproj(h):
                    wq_sb = wqp.tile([128, KD, HD], BF16, tag="wqh")
                    nc.sync.dma_start(wq_sb[:],
                                      wq_v[:, :, h * HD:(h + 1) * HD])
                    pq = pproj.tile([128, TLOC], F32, tag="pp")
                    for k in range(KD):
                        nc.tensor.matmul(
                            pq[:], lhsT=wq_sb[:, k, :], rhs=xt[:, k, :],
                            start=(k == 0), stop=(k == KD - 1),
                        )
                    rope_apply(rope_pool, qT_sb[:, h, :], pq)

                def attention(kvh):
                    # raw gathered blocks: lo = cores 0-3, hi = cores 4-7.
                    # One strided DMA each for K and V (vs 16 block DMAs).
                    kTr = kva.tile([128, NC, TLOC], BF16, tag="kTr")
                    vr = kva.tile([128, NC * 4, HD], BF16, tag="vr")
                    nc.sync.dma_start(kTr[:], kv_gk[:, :, kvh, :])
                    nc.sync.dma_start(
                        vr[:], kv_gv[:, :, :, HKV + kvh, :])
                    # in-place batch-select into the lo half (Pool engine):
                    # lo = lo*s0 + hi*s1  with (s0,s1) = (1,0) or (0,1).
                    lo_k = kTr[:, 0:GSZ, :]
                    hi_k = kTr[:, GSZ:NC, :]
                    nc.vector.tensor_scalar_mul(lo_k, lo_k, s0)
                    nc.vector.scalar_tensor_tensor(
                        lo_k, hi_k, s1, lo_k, MUL, ADD)
                    lo_v = vr[:, 0:GSZ * 4, :]
                    hi_v = vr[:, GSZ * 4:NC * 4, :]
                    nc.vector.tensor_scalar_mul(lo_v, lo_v, s0)
                    nc.vector.scalar_tensor_tensor(
                        lo_v, hi_v, s1, lo_v, MUL, ADD)

                    for q in range(GPH):
                        qh = kvh * GPH + q
                        po = po_ps.tile([128, TLOC], F32, tag="po")
                        # denominator: pairwise fold of whole e-tiles
                        # ([128,1024] adds) — same element work as a
                        # per-tile tree but 8 DVE ops/head instead of 15.
                        prev_e = None
                        pend = None
                        pairs = []
                        for sg in range(NSG):
                            ps = pe_ps.tile([128, SGRP * TLOC], F32, tag="ps")
                            for j in range(SGRP):
                                st = sg * SGRP + j
                                blk, w = st // 4, st % 4
                                nc.tensor.matmul(
                                    ps[:, j * TLOC:(j + 1) * TLOC],
                                    lhsT=kTr[:, blk, w * 128:(w + 1) * 128],
                                    rhs=qT_sb[:, qh, :],
                                    start=True, stop=True,
                                )
                            e = epool.tile([128, SGRP * TLOC], BF16, tag="e")
                            nc.scalar.activation(e[:], ps[:], AF.Exp,
                                                 scale=scale)
                            # PV runs one group late so the next group's
                            # scores sit between exp and PV on the in-order
                            # PE queue (fills the exp-latency bubble).
                            if pend is not None:
                                pe_, psg = pend
                                for j in range(SGRP):
                                    st = psg * SGRP + j
                                    nc.tensor.matmul(
                                        po[:],
                                        lhsT=vr[:, st, :],
                                        rhs=pe_[:, j * TLOC:(j + 1) * TLOC],
                                        start=(psg == 0 and j == 0),
                                        stop=False,
                                    )
                            pend = (e, sg)
                            if sg % 2 == 0:
                                prev_e = e
                            else:
                                t = dpool.tile([128, SGRP * TLOC], BF16,
                                               tag="dpair", bufs=5, name="dp")
                                nc.vector.tensor_add(t[:], prev_e[:], e[:])
                                pairs.append(t)
                        pe_, psg = pend
                        for j in range(SGRP):
                            st = psg * SGRP + j
                            nc.tensor.matmul(
                                po[:],
                                lhsT=vr[:, st, :],
                                rhs=pe_[:, j * TLOC:(j + 1) * TLOC],
                                start=False,
                                stop=(j == SGRP - 1),
                            )
                        quads = []
                        for i in range(2):
                            qd = dpool.tile([128, SGRP * TLOC], BF16,
                                            tag="dquad", bufs=3, name="dq")
                            nc.vector.tensor_add(qd[:], pairs[2 * i][:],
                                                 pairs[2 * i + 1][:])
                            quads.append(qd)
                        tot = dpool.tile([128, SGRP * TLOC], BF16,
                                         tag="dtot", bufs=2, name="dto")
                        nc.vector.tensor_add(tot[:], quads[0][:], quads[1][:])
                        den = dpool.tile([128, TLOC], BF16, tag="dden",
                                         bufs=2, name="dde")
                        nc.vector.tensor_add(den[:], tot[:, 0:TLOC],
                                             tot[:, TLOC:2 * TLOC])
                        dall = dpool.tile([128, TLOC], F32, tag="dall",
                                          bufs=2)
                        nc.gpsimd.partition_all_reduce(
                            dall[:], den[:], channels=128,
                            reduce_op=bass_isa.ReduceOp.add)
                        rcp = dpool.tile([128, TLOC], F32, tag="rcp", bufs=2)
                        nc.vector.reciprocal_approx_fast(rcp[:], dall[:])
                        nc.vector.tensor_mul(
                            oT_sb[:, qh, :], po[:], rcp[:])

                NPRE = 24
                for h in range(NPRE):
                    qproj(h)
                tail = list(range(NPRE, H))
                for kvh in range(HKV):
                    attention(kvh)
                    if kvh < 4 and tail:
                        qproj(tail.pop(0))
                        qproj(tail.pop(0))

            # ---------- phase O: local o_proj ----------
            DC = 512
            KO = H  # 32 k-tiles (head-major rows of wo)
            with tc.tile_pool(name="wop", bufs=2) as wop, \
                 tc.tile_pool(name="osb", bufs=3) as osb_pool, \
                 tc.tile_pool(name="po3", bufs=4, space="PSUM") as po3:
                for dch in range(D // DC):
                    wo_sb = wop.tile([128, KO, DC], BF16, tag="wo_sb")
                    for kg in range(0, KO, 4):
                        nc.sync.dma_start(
                            wo_sb[:, kg:kg + 4, :],
                            wo_v[:, kg:kg + 4, dch * DC:(dch + 1) * DC])
                    for tt in range(TLOC // 128):
                        pso = po3.tile([128, DC], F32, tag="pso")
                        for k in range(KO):
                            nc.tensor.matmul(
                                pso[:],
                                lhsT=oT_sb[:, k, tt * 128:(tt + 1) * 128],
                                rhs=wo_sb[:, k, :],
                                start=(k == 0), stop=(k == KO - 1),
                            )
                        o_out = osb_pool.tile([128, DC], F32, tag="osb")
                        nc.vector.tensor_copy(o_out[:], pso[:])
                        nc.sync.dma_start(
                            out[tt * 128:(tt + 1) * 128,
                                dch * DC:(dch + 1) * DC],
                            o_out[:],
                        )

    nc.compile()
    return nc


# ------------------------------------------------------------------
# host-side input prep
# ------------------------------------------------------------------

def _rope_perm(n_heads_cols: int, HD: int) -> np.ndarray:
    """Column permutation: per head, evens first then odds."""
    idx = np.arange(n_heads_cols)
    h = idx // HD
    j = idx % HD
    old = np.where(j < HD // 2, 2 * j, 2 * (j - HD // 2) + 1)
    return h * HD + old


def make_in_maps(inputs: dict, cfg: Cfg = FULL):
    B, T, D, HD, NC, TLOC = cfg.B, cfg.T, cfg.D, cfg.HD, cfg.NC, cfg.TLOC
    bf = ml_dtypes.bfloat16

    x = np.asarray(inputs["x"], np.float32).reshape(B * T, D)
    xT = np.ascontiguousarray(x.T).astype(bf)      # [D, TOK]

    wq = np.asarray(inputs["wq"], np.float32)
    wk = np.asarray(inputs["wk"], np.float32)
    wv = np.asarray(inputs["wv"], np.float32)
    wo = np.asarray(inputs["wo"], np.float32)

    wq_p = np.ascontiguousarray(wq[:, _rope_perm(wq.shape[1], HD)]).astype(bf)
    wk_p = np.ascontiguousarray(wk[:, _rope_perm(wk.shape[1], HD)]).astype(bf)
    wv_b = np.ascontiguousarray(wv).astype(bf)
    wo_b = np.ascontiguousarray(wo).astype(bf)

    cos = np.asarray(inputs["freqs_cos"], np.float32)   # [T, 64]
    sin = np.asarray(inputs["freqs_sin"], np.float32)
    cosT = np.ascontiguousarray(cos.T)                  # [64, T]
    sinT = np.ascontiguousarray(sin.T)

    in_maps = []
    for c in range(NC):
        t0 = c * TLOC              # global token start
        p0 = t0 % T                # position within batch
        batch = c // (NC // B)
        bsel = np.zeros((128, 2), np.float32)
        bsel[:, batch] = 1.0
        in_maps.append({
            "xT": np.ascontiguousarray(xT[:, t0:t0 + TLOC]),
            "wq": wq_p,
            "wk": wk_p,
            "wv": wv_b,
            "wo": wo_b,
            "cosT": np.ascontiguousarray(cosT[:, p0:p0 + TLOC]),
            "sinT": np.ascontiguousarray(sinT[:, p0:p0 + TLOC]),
            "bsel": bsel,
        })
    return in_maps


_CACHE: dict = {}


def kernel(**inputs) -> np.ndarray:
    cfg = FULL
    sp = inputs.get("start_pos", 0)
    sp = int(np.asarray(sp).reshape(-1)[0]) if np.asarray(sp).size else 0
    assert sp == 0, f"kernel only supports start_pos=0, got {sp}"

    from concourse.bass_utils import run_bass_kernel_spmd

    if "nc" not in _CACHE:
        _CACHE["nc"] = build_nc(cfg)
    nc = _CACHE["nc"]

    in_maps = make_in_maps(inputs, cfg)
    res = run_bass_kernel_spmd(nc, in_maps, list(range(cfg.NC)))
    outs = [res.results[c]["out"] for c in range(cfg.NC)]
    full = np.concatenate(outs, axis=0)          # [TOK, D]
    return full.reshape(cfg.B, cfg.T, cfg.D).astype(np.float32)


if __name__ == "__main__":
    nc = build_nc()
    n = sum(len(bb.instructions) for bb in nc.m.functions[0].blocks)
    print("built", n, "instructions")
